# revision 48
# baseline (speedup 1.0000x reference)
"""Trainium2 Bass kernel for nn_CausalSelfAttention_5411658793445 (v2).

Token-sharded SPMD over 8 cores. Per core: 330 query tokens; K/V for the
current block exchanged via AllGather; every core attends over the full
kept window (2640 prior + 2640 current KV positions, dense l-tiling).

v2 structural changes vs v1:
  - exp batched over 3 l-tiles per ACTIVATE (amortizes ~300ns ACT overhead)
  - dense pass-2 l-grid (21 tiles, not 24): K gathered unpadded, V shards
    compacted into a partition-major dense DRAM grid after the AllGather
  - denominator via separate 1-column ones matmul with masked last-tile
    ones vector (no padding corrections anywhere)
  - V projection interleaved into pass-1 emission as PE filler so the
    tensor engine stays busy while ACT paces the exp stream
  - f32r (full-rate) matmuls for sum-of-squares and broadcast helpers
  - host-side weight/prior re-layout for contiguous, low-descriptor DMAs
  - output projection streamed directly at the tail, psum -> sbuf on ACT
"""

import math
from contextlib import ExitStack

import numpy as np
import ml_dtypes

NC = 8
DIM, NH, HD = 1536, 12, 128
HALF = 64
H, W = 22, 40
FRAME = H * W            # 880
S_TOTAL = 2640
SC = S_TOTAL // NC       # 330
ST = 110                 # token sub-chunk for [t,*] psum tiles (330 = 3*110)
NK = DIM // 128          # 12 contraction chunks
EPS = 1e-6
CT = 22
CH = 21
CW = 21
GRP = 3                  # l-tiles per exp group

_BF16 = ml_dtypes.bfloat16
_cache: dict = {}


def _build_theta(freqs_angle, cs):
    start_frame = cs // FRAME
    nf = S_TOTAL // FRAME
    t = freqs_angle[start_frame:start_frame + nf, :CT]
    h = freqs_angle[:H, CT:CT + CH]
    w = freqs_angle[:W, CT + CH:CT + CH + CW]
    tf = np.broadcast_to(t[:, None, None, :], (nf, H, W, CT))
    hf = np.broadcast_to(h[None, :, None, :], (nf, H, W, CH))
    wf = np.broadcast_to(w[None, None, :, :], (nf, H, W, CW))
    return np.concatenate([tf, hf, wf], axis=-1).reshape(nf * H * W, HALF)


def _v_pieces(shard):
    """Split the flat gathered-V token space into partition-aligned pieces.

    Returns [(c, tok0, n, p0, lt)]: shard c tokens [tok0, tok0+n) land at
    partitions [p0, p0+n) of l-tile lt of the dense [128, NCT, HD] grid.
    """
    pieces = []
    for c in range(NC):
        flat = c * shard
        left = shard
        tok = 0
        while left:
            p0 = flat % 128
            lt = flat // 128
            n = min(128 - p0, left)
            pieces.append((c, tok, n, p0, lt))
            flat += n
            tok += n
            left -= n
    return pieces


def _build_program(n_prior):
    import concourse.bass as bass  # noqa: F401
    import concourse.tile as tile
    from concourse import bacc, mybir
    from concourse.masks import make_identity

    f32 = mybir.dt.float32
    f32r = mybir.dt.float32r
    bf16 = mybir.dt.bfloat16
    Act = mybir.ActivationFunctionType
    Alu = mybir.AluOpType

    NPT = -(-n_prior // 128)         # prior l-tiles (21)
    NCT = -(-S_TOTAL // 128)         # current dense l-tiles (21)
    np_pad = NPT * 128
    PVALID = n_prior - 128 * (NPT - 1)   # valid rows in last prior tile (80)
    CVALID = S_TOTAL - 128 * (NCT - 1)   # valid rows in last current tile (80)
    PG = -(-NPT // GRP)              # prior exp groups per head (7)
    CG = -(-NCT // GRP)              # current exp groups per head (7)
    sm_scale = 1.0 / math.sqrt(HD)

    nc = bacc.Bacc("TRN2", target_bir_lowering=False, debug=False,
                   num_devices=NC)

    xs_p = nc.dram_tensor("xs_p", [128, NK, SC], bf16, kind="ExternalInput").ap()
    cc2 = nc.dram_tensor("cc2", [128, SC], f32, kind="ExternalInput").ap()
    ss2 = nc.dram_tensor("ss2", [128, SC], f32, kind="ExternalInput").ap()
    wq_p = nc.dram_tensor("wq_p", [NH, 128, NK, 128], bf16, kind="ExternalInput").ap()
    wk_p = nc.dram_tensor("wk_p", [NH, 128, NK, 128], bf16, kind="ExternalInput").ap()
    wv_p = nc.dram_tensor("wv_p", [128, NK, DIM], bf16, kind="ExternalInput").ap()
    wo_p = nc.dram_tensor("wo_p", [NH, 128, 3, 512], bf16, kind="ExternalInput").ap()
    bq2 = nc.dram_tensor("bq2", [HD, NH], f32, kind="ExternalInput").ap()
    bk2 = nc.dram_tensor("bk2", [HD, NH], f32, kind="ExternalInput").ap()
    gq2 = nc.dram_tensor("gq2", [HD, NH], f32, kind="ExternalInput").ap()
    gk2 = nc.dram_tensor("gk2", [HD, NH], f32, kind="ExternalInput").ap()
    bv1 = nc.dram_tensor("bv1", [1, DIM], bf16, kind="ExternalInput").ap()
    bo1 = nc.dram_tensor("bo1", [1, DIM], bf16, kind="ExternalInput").ap()
    pswT = nc.dram_tensor("pswT", [HD, HD], bf16, kind="ExternalInput").ap()
    priorKT = nc.dram_tensor("priorKT", [NH, HD, np_pad], bf16,
                             kind="ExternalInput").ap()
    priorVp = nc.dram_tensor("priorVp", [NH, 128, NPT, 130], bf16,
                             kind="ExternalInput").ap()
    dmask = nc.dram_tensor("dmask", [128, 2], bf16, kind="ExternalInput").ap()
    out = nc.dram_tensor("out", [SC, DIM], f32, kind="ExternalOutput").ap()
    import os
    DBG = bool(int(os.environ.get("KERNEL_DBG", "0")))
    if DBG:
        dbg_rrb = nc.dram_tensor("dbg_rrb", [128, SC], f32,
                                 kind="ExternalOutput").ap()
        dbg_esc = nc.dram_tensor("dbg_esc", [128, GRP * SC], f32,
                                 kind="ExternalOutput").ap()
        dbg_p1 = nc.dram_tensor("dbg_p1", [128, NH * 3 * 132], f32,
                                kind="ExternalOutput").ap()

    def r32(ap):
        return ap.bitcast(f32r)

    with tile.TileContext(nc, trace_sim=False) as tc, ExitStack() as ctx:
        consts = ctx.enter_context(tc.tile_pool(name="consts", bufs=1))
        wstr = ctx.enter_context(tc.tile_pool(name="wstr", bufs=2))
        xpool = ctx.enter_context(tc.tile_pool(name="xpool", bufs=1))
        acts = ctx.enter_context(tc.tile_pool(name="acts", bufs=1))
        sqp = ctx.enter_context(tc.tile_pool(name="sqp", bufs=2))
        escp = ctx.enter_context(tc.tile_pool(name="escp", bufs=2))
        kvs = ctx.enter_context(tc.tile_pool(name="kvs", bufs=2))
        g2p = ctx.enter_context(tc.tile_pool(name="g2p", bufs=2))
        smal = ctx.enter_context(tc.tile_pool(name="smal", bufs=4))
        dram = ctx.enter_context(tc.tile_pool(name="dram", bufs=1, space="DRAM"))
        # PSUM: big = 2 slots x 3 banks (score groups / proj accumulators /
        # tail out-proj chains); pone = pos accumulator (1 bank);
        # vone = V-filler chain / pss / transposes (1 bank, 2 slots... )
        big = ctx.enter_context(tc.tile_pool(name="big", bufs=2, space="PSUM"))
        pone = ctx.enter_context(tc.tile_pool(name="pone", bufs=1, space="PSUM"))
        vone = ctx.enter_context(tc.tile_pool(name="vone", bufs=1, space="PSUM"))

        # ---------------- constants ----------------
        _constv_cache = {}

        def constv(val):
            if val not in _constv_cache:
                t = consts.tile([128, 1], f32, name=f"cv_{len(_constv_cache)}")
                nc.vector.memset(t, val)
                _constv_cache[val] = t
            return _constv_cache[val]

        ident = consts.tile([128, 128], bf16)
        make_identity(nc, ident)
        ones_col_b = consts.tile([128, 1], bf16)
        nc.vector.memset(ones_col_b, 1.0)
        ones_row_b = consts.tile([1, 128], bf16)
        nc.vector.memset(ones_row_b, 1.0)
        ones_row_f = consts.tile([1, 128], f32)
        nc.vector.memset(ones_row_f, 1.0)
        dden_full = consts.tile([128, 1], bf16)
        nc.vector.memset(dden_full, 1.0)
        dmask_sb = consts.tile([128, 2], bf16)
        nc.sync.dma_start(dmask_sb, dmask)
        dden_p = dmask_sb[:, 0:1]
        dden_c = dmask_sb[:, 1:2]
        psw_sb = consts.tile([HD, HD], bf16)
        nc.sync.dma_start(psw_sb, pswT)
        cc_sb = consts.tile([128, SC], f32)
        ss_sb = consts.tile([128, SC], f32)
        nc.sync.dma_start(cc_sb, cc2)
        nc.sync.dma_start(ss_sb, ss2)
        bq_sb = consts.tile([HD, NH], f32)
        bk_sb = consts.tile([HD, NH], f32)
        gq_sb = consts.tile([HD, NH], f32)
        gk_sb = consts.tile([HD, NH], f32)
        nc.sync.dma_start(bq_sb, bq2)
        nc.sync.dma_start(bk_sb, bk2)
        nc.sync.dma_start(gq_sb, gq2)
        nc.sync.dma_start(gk_sb, gk2)
        bqg = consts.tile([HD, NH], f32)
        bkg = consts.tile([HD, NH], f32)
        nc.vector.tensor_mul(bqg, bq_sb, gq_sb)
        nc.vector.tensor_mul(bkg, bk_sb, gk_sb)
        bv_sb = consts.tile([1, DIM], bf16)
        bo_sb = consts.tile([1, DIM], bf16)
        nc.sync.dma_start(bv_sb, bv1)
        nc.sync.dma_start(bo_sb, bo1)
        zpad = consts.tile([64, DIM], bf16)
        nc.vector.memset(zpad, 0.0)

        # ---------------- x ----------------
        xs = xpool.tile([128, NK, SC], bf16)
        nc.sync.dma_start(xs, xs_p)

        # ---------------- internal DRAM ----------------
        k_cc_in = dram.tile([NH, HD, SC], bf16)
        v_cc_in = dram.tile([SC, DIM], bf16)
        kg = dram.tile([NC, NH, HD, SC], bf16, addr_space="Shared")
        vg = dram.tile([NC, SC, DIM], bf16, addr_space="Shared")
        vgd = dram.tile([NH, 128, NCT, 130], bf16)
        rgroups = [list(range(NC))]

        # prefetch prior KV for head 0 (kvs pool) before any compute
        def load_prior(h):
            pkh = kvs.tile([128, np_pad], bf16, tag="kload", name=f"pk_{h}")
            nc.sync.dma_start(pkh, priorKT[h])
            pvh = kvs.tile([128, NPT, 130], bf16, tag="vload", name=f"pv_{h}")
            nc.sync.dma_start(pvh, priorVp[h])
            return pkh, pvh

        prior_bufs = {0: load_prior(0)}

        # ------------- q/k projection (dense) -------------
        def qk_projection(w_dram, b_sb, g_sb, bg_sb, name):
            raw = acts.tile([128, NH, SC], bf16, tag="raw", name=f"raw_{name}")
            pss = vone.tile([128, 512], f32, tag="vch", name=f"pss_{name}")
            for m in range(NH):
                wm = wstr.tile([128, NK, 128], bf16, tag="wm",
                               name=f"wm_{name}_{m}")
                nc.sync.dma_start(wm, w_dram[m])
                ps = big.tile([128, GRP, 512], f32, tag="spp",
                              name=f"pj_{name}_{m}")
                for kk in range(NK):
                    nc.tensor.matmul(
                        ps[:, 0, :SC], wm[:, kk, :], xs[:, kk, :],
                        start=(kk == 0), stop=(kk == NK - 1))
                nc.scalar.activation(raw[:, m, :], ps[:, 0, :SC], Act.Identity,
                                     bias=bg_sb[:, m:m + 1],
                                     scale=g_sb[:, m:m + 1])
                sq = sqp.tile([128, SC], bf16, tag="sq")
                nc.scalar.activation(sq, ps[:, 0, :SC], Act.Square,
                                     bias=b_sb[:, m:m + 1])
                nc.tensor.matmul(pss[0:1, :SC], ones_col_b, sq,
                                 start=(m == 0), stop=(m == NH - 1))
            r1 = smal.tile([1, SC], f32, tag="r1")
            nc.scalar.activation(r1, pss[0:1, :SC], Act.Sqrt,
                                 scale=1.0 / DIM, bias=constv(EPS)[0:1])
            rr = smal.tile([1, SC], f32, tag="rr")
            nc.vector.reciprocal(rr, r1)
            rrb = pone.tile([128, 3, 160], f32, tag="pos", name=f"rrb_{name}")
            rrf = rrb.rearrange("p a b -> p (a b)")
            nc.tensor.matmul(rrf[:, :SC], ones_row_f, rr,
                             start=True, stop=True)
            ccr = sqp.tile([128, SC], f32, tag="ccr", name=f"ccr_{name}")
            ssr = sqp.tile([128, SC], f32, tag="ssr", name=f"ssr_{name}")
            nc.vector.tensor_mul(ccr, cc_sb, rrf[:, :SC])
            nc.vector.tensor_mul(ssr, ss_sb, rrf[:, :SC])
            if DBG and name == "q":
                nc.sync.dma_start(dbg_rrb, rrb)
            return raw, ccr, ssr

        def rope_chunk(raw, ccr, ssr, m, dst_ap, name):
            # dst = raw*ccr + swap_halves(raw)*ssr (swap via PE matmul)
            pw = big.tile([128, GRP, 512], f32, tag="spp", name=f"sw_{name}_{m}")
            nc.tensor.matmul(pw[:, 0, :SC], psw_sb, raw[:, m, :],
                             start=True, stop=True)
            m1 = sqp.tile([128, SC], f32, tag="m1")
            nc.vector.tensor_mul(m1, raw[:, m, :], ccr)
            m2 = sqp.tile([128, SC], f32, tag="m2")
            nc.vector.tensor_mul(m2, pw[:, 0, :SC], ssr)
            nc.vector.tensor_add(dst_ap, m1, m2)

        # ---------- K projection + AllGather ----------
        raw_k, ccr_k, ssr_k = qk_projection(wk_p, bk_sb, gk_sb, bkg, "k")
        kn = acts.tile([128, NH, SC], bf16, tag="kn")
        for m in range(NH):
            rope_chunk(raw_k, ccr_k, ssr_k, m, kn[:, m, :], "k")
        for m in range(NH):
            nc.gpsimd.dma_start(k_cc_in[m], kn[:, m, :])
        nc.gpsimd.collective_compute(
            "AllGather", Alu.bypass, replica_groups=rgroups,
            ins=[k_cc_in.opt()], outs=[kg.opt()])

        # ---------- V projection (dense, right after K so AG-V starts
        # early) -- direct [t, d] production in 128-row token chunks, then
        # three contiguous shard DMAs + AllGather + dense-grid compaction.
        wvs_all = xpool.tile([128, NK, DIM], bf16, tag="wvs")
        nc.sync.dma_start(wvs_all, wv_p)
        vt = acts.tile([128, 3, DIM], bf16, tag="vt")
        VROWS = [128, 128, SC - 256]
        for oc in range(3):
            for tc in range(3):
                rows = VROWS[tc]
                pv = vone.tile([128, 512], f32, tag="vch",
                               name=f"pv_{oc}_{tc}")
                for kk in range(NK):
                    nc.tensor.matmul(
                        pv[:rows, :], xs[:, kk, tc * 128:tc * 128 + rows],
                        wvs_all[:, kk, oc * 512:(oc + 1) * 512],
                        start=(kk == 0), stop=False)
                nc.tensor.matmul(pv[:rows, :], ones_row_b[:, :rows],
                                 bv_sb[:, oc * 512:(oc + 1) * 512],
                                 start=False, stop=True)
                nc.vector.tensor_copy(
                    vt[:rows, tc, oc * 512:(oc + 1) * 512], pv[:rows, :])
        for tc in range(3):
            rows = VROWS[tc]
            nc.gpsimd.dma_start(v_cc_in[tc * 128:tc * 128 + rows, :],
                                vt[:rows, tc, :])
        nc.gpsimd.collective_compute(
            "AllGather", Alu.bypass, replica_groups=rgroups,
            ins=[v_cc_in.opt()], outs=[vg.opt()])

        def emit_v_compaction():
            # compact gathered V into the dense partition-major grid;
            # emitted after pass-1 so these AG-blocked DMAs don't clog the
            # queue ahead of pass-1's weight/KV loads.
            for (c, tok0, n, p0, lt) in _v_pieces(SC):
                nc.sync.dma_start(
                    vgd[:, p0:p0 + n, lt, 0:HD].rearrange("h p d -> p h d"),
                    vg[c, tok0:tok0 + n, :].rearrange("p (h d) -> p h d",
                                                      h=NH))
            # zero the dense-grid pad rows (last tile)
            if CVALID < 128:
                nc.sync.dma_start(
                    vgd[:, CVALID:128, NCT - 1, 0:HD].rearrange(
                        "h p d -> p h d"),
                    zpad[0:128 - CVALID, :].rearrange("p (h d) -> p h d",
                                                      h=NH))

        # ---------- Q projection ----------
        raw_q, ccr_q, ssr_q = qk_projection(wq_p, bq_sb, gq_sb, bqg, "q")
        qn = acts.tile([128, NH, SC], bf16, tag="qn")
        for m in range(NH):
            rope_chunk(raw_q, ccr_q, ssr_q, m, qn[:, m, :], "q")
        v_units = []

        # wo fully resident (reuses the wv slot; loads during pass-1)
        wo_all = xpool.tile([128, NH, 3, 512], bf16, tag="wvs", name="wo_all")
        nc.sync.dma_start(wo_all, wo_p.rearrange("h ki oc c -> ki h oc c"))

        # ---------- attention ----------
        part1 = acts.tile([128, NH, 3, 132], f32, tag="part1")
        oT = acts.tile([128, NH, SC], bf16, tag="oT")

        def attn_all(specs, fillers, phase):
            """Run one softmax pass over all heads, software-pipelined.

            Scores+exp of pipeline step i+1 are emitted before the AV
            matmuls of step i, across head boundaries, so the in-order PE
            queue always has exp-independent work ahead of each
            exp-dependent AV block. `fillers` are exp-free PE work units
            popped one per step to absorb the ACT-vs-PE rate gap.
            """
            flat = []
            for sp in specs:
                for g in range(sp["n_groups"]):
                    flat.append((sp, g))
            esc_q = []

            def emit_scores(i):
                sp, g = flat[i]
                g0 = g * GRP
                gn = min(GRP, sp["n_tiles"] - g0)
                spp = big.tile([128, GRP, 512], f32, tag="spp",
                               name=f"sc_{phase}_{sp['h']}_{g}")
                for j in range(gn):
                    nc.tensor.matmul(spp[:, j, :SC], sp["k_tile"](g0 + j),
                                     qn[:, sp["h"], :], start=True, stop=True)
                esc = escp.tile([128, GRP, SC], bf16, tag="esc")
                nc.scalar.activation(esc[:, :gn, :], spp[:, :gn, :SC],
                                     Act.Exp, scale=float(sm_scale))
                if DBG and phase == "p" and i == 0:
                    dbge = acts.tile([128, GRP * SC], f32, tag="dbge")
                    nc.vector.tensor_copy(
                        dbge.rearrange("p (a b) -> p a b", a=GRP),
                        esc)
                    nc.sync.dma_start(dbg_esc, dbge)
                esc_q.append((sp, g, esc))

            def emit_av():
                sp, g, esc = esc_q.pop(0)
                if g == 0:
                    # prefetch the next head's KV now: the previous head's
                    # reads are all emitted, so the pool slot is reclaimable
                    if sp.get("pre") is not None:
                        sp["pre"]()
                    sp["pos"] = pone.tile([128, 3, 160], f32, tag="pos",
                                          name=f"pos_{phase}_{sp['h']}")
                pos = sp["pos"]
                g0 = g * GRP
                gn = min(GRP, sp["n_tiles"] - g0)
                for j in range(gn):
                    lt = g0 + j
                    last = lt == sp["n_tiles"] - 1
                    vt_ap = sp["v_tile"](lt)   # [128, 129]: V | ones-col
                    for si in range(3):
                        # a psum-bank `start` zeroes the whole bank, so only
                        # the very first matmul of the bank may set it
                        nc.tensor.matmul(
                            pos[:ST, si, 0:129],
                            esc[:, j, si * ST:(si + 1) * ST], vt_ap,
                            start=(lt == 0 and si == 0), stop=last,
                            skip_group_check=True)
                if g == sp["n_groups"] - 1 and sp.get("post") is not None:
                    sp["post"](sp)

            emit_scores(0)
            for i in range(1, len(flat) + 1):
                if fillers:
                    fillers.pop(0)()
                if i < len(flat):
                    emit_scores(i)
                emit_av()

        # ---- pass 1: prior KV (AllGathers in flight) ----
        def p1_pre(h):
            def f():
                if h + 1 < NH:
                    prior_bufs[h + 1] = load_prior(h + 1)
            return f

        def p1_post(sp):
            pos = sp["pos"]
            h = sp["h"]
            for si in range(3):
                nc.vector.tensor_copy(part1[:ST, h, si, 0:129],
                                      pos[:ST, si, 0:129])

        p1_specs = []
        for h in range(NH):
            p1_specs.append(dict(
                h=h, n_tiles=NPT, n_groups=PG,
                k_tile=lambda lt, h=h: prior_bufs[h][0][:, lt * 128:
                                                        (lt + 1) * 128],
                v_tile=lambda lt, h=h: prior_bufs[h][1][:, lt, 0:129],
                pre=p1_pre(h), post=p1_post))
        attn_all(p1_specs, v_units, "p")
        emit_v_compaction()
        if DBG:
            nc.sync.dma_start(
                dbg_p1, part1.rearrange("p a b c -> p (a b c)"))
        # drain any remaining V units (ensures AG-V is triggered)
        while v_units:
            v_units.pop(0)()

        # ---- pass 2: gathered current KV (dense grid) ----
        def load_g2(h):
            kgh = g2p.tile([128, NCT * 128], bf16, tag="kload2",
                           name=f"kg_{h}")
            nc.sync.dma_start(
                kgh[:, 0:S_TOTAL].rearrange("p (c t) -> p c t", c=NC),
                kg[:, h].rearrange("c p t -> p c t"))
            if NCT * 128 > S_TOTAL:
                nc.vector.memset(kgh[:, S_TOTAL:], 0.0)
            vgh = g2p.tile([128, NCT, 130], bf16, tag="vload2",
                           name=f"vg_{h}")
            nc.sync.dma_start(vgh[:, :, 0:HD], vgd[h, :, :, 0:HD])
            nc.vector.memset(vgh[:, :, 128:129], 1.0)
            nc.vector.tensor_copy(vgh[:, NCT - 1, 128:129], dden_c)
            return kgh, vgh

        g2_bufs = {0: load_g2(0)}

        def p2_pre(h):
            def f():
                if h + 1 < NH:
                    g2_bufs[h + 1] = load_g2(h + 1)
            return f

        def p2_post(sp):
            # combine partials + divide (all reads of pos first, since the
            # transpose psum reuses the pos slot), then transpose to [d, t]
            pos = sp["pos"]
            h = sp["h"]
            odivs = []
            for si in range(3):
                nsb = smal.tile([128, 132], f32, tag="nsb")
                nc.vector.tensor_add(nsb[:ST, 0:129], pos[:ST, si, 0:129],
                                     part1[:ST, h, si, 0:129])
                rcp = smal.tile([128, 1], f32, tag="rcp")
                nc.vector.reciprocal(rcp[:ST, :], nsb[:ST, 128:129])
                odiv = smal.tile([128, 128], bf16, tag="odiv",
                                 name=f"odiv_{h}_{si}")
                nc.scalar.activation(odiv[:ST, :], nsb[:ST, 0:128],
                                     Act.Copy, scale=rcp[:ST, 0:1])
                odivs.append(odiv)
            for si in range(3):
                ptr = pone.tile([128, 3, 160], bf16, tag="pos",
                                name=f"ptr_{h}_{si}")
                ptrf = ptr.rearrange("p a b -> p (a b)")
                nc.tensor.transpose(ptrf[:, :ST], odivs[si][:ST, :],
                                    ident[:ST, :ST])
                nc.vector.tensor_copy(oT[:, h, si * ST:(si + 1) * ST],
                                      ptrf[:, :ST])

        p2_specs = []
        for h in range(NH):
            p2_specs.append(dict(
                h=h, n_tiles=NCT, n_groups=CG,
                k_tile=lambda lt, h=h: g2_bufs[h][0][:, lt * 128:
                                                     (lt + 1) * 128],
                v_tile=lambda lt, h=h: g2_bufs[h][1][:, lt, 0:129],
                pre=p2_pre(h), post=p2_post))
        attn_all(p2_specs, [], "c")

        # ---------- output projection (tail, weights resident) ----------
        for oc in range(3):
            po = big.tile([128, GRP, 512], f32, tag="spp", name=f"po_{oc}")
            for h in range(NH):
                for tci in range(3):
                    nc.tensor.matmul(
                        po[:ST, tci, :], oT[:, h, tci * ST:(tci + 1) * ST],
                        wo_all[:, h, oc, :], start=(h == 0), stop=False)
            for tci in range(3):
                nc.tensor.matmul(
                    po[:ST, tci, :], ones_row_b[:, :ST],
                    bo_sb[:, oc * 512:(oc + 1) * 512],
                    start=False, stop=True)
            for tci in range(3):
                ob = sqp.tile([128, 512], f32, tag="ob",
                              name=f"ob_{oc}_{tci}")
                nc.scalar.activation(ob[:ST, :], po[:ST, tci, :], Act.Copy)
                nc.sync.dma_start(
                    out[tci * ST:(tci + 1) * ST, oc * 512:(oc + 1) * 512],
                    ob[:ST, :])

    nc.compile()
    return nc


def _prep(inputs):
    x = np.asarray(inputs["x"], np.float32)
    freqs_angle = np.asarray(inputs["freqs_angle"], np.float32)
    prior_k = np.asarray(inputs["prior_k"], np.float32)
    prior_v = np.asarray(inputs["prior_v"], np.float32)
    cs = int(np.asarray(inputs["current_start"]))

    block = 3 * FRAME
    block_end = (cs // block + 1) * block
    keep_from = max(0, block_end - 6 * FRAME)
    keep_size = min(cs + S_TOTAL - keep_from, prior_k.shape[0] + S_TOTAL)
    n_prior = keep_size - S_TOTAL
    p0 = prior_k.shape[0] - n_prior
    NPT = -(-n_prior // 128)
    np_pad = NPT * 128

    perm = np.concatenate(
        [h * HD + np.concatenate([np.arange(0, HD, 2), np.arange(1, HD, 2)])
         for h in range(NH)])

    Wq = np.asarray(inputs["Wq"], np.float32)[perm]
    Wk = np.asarray(inputs["Wk"], np.float32)[perm]
    Wv = np.asarray(inputs["Wv"], np.float32)
    Wo = np.asarray(inputs["Wo"], np.float32)

    # wq_p / wk_p: [NH(m), 128(ki), NK(ko), 128(col)]; W row=outdim col=indim
    # device matmul: stationary wm[:, kk, :] = W^T chunk [ki, m-cols]
    def qk_prep(Wm):
        WT = Wm.T.reshape(NK, 128, NH, 128)           # [ko, ki, m, col]
        return np.ascontiguousarray(
            WT.transpose(2, 1, 0, 3)).astype(_BF16)    # [m, ki, ko, col]

    wq_prep = qk_prep(Wq)
    wk_prep = qk_prep(Wk)
    # wv_p: [128(ki), NK(ko), 512]; chunk for oc: [:, kk, oc*512...] is
    # WvT[kk*128:(kk+1)*128 rows?? -> WvT reshaped
    WvT = Wv.T.reshape(NK, 128, DIM)                   # [ko, ki, outcol]
    wv_prep = np.ascontiguousarray(
        WvT.transpose(1, 0, 2)).astype(_BF16)          # [ki, ko, 1536]
    WoT = Wo.T.reshape(NK, 128, 3, 512)                # [h, ki, oc, col]
    wo_prep = np.ascontiguousarray(
        WoT.transpose(0, 1, 2, 3)).astype(_BF16)       # [h(=ko), ki, oc, col]

    def two(vec, p=None):
        v = np.asarray(vec, np.float32)
        if p is not None:
            v = v[p]
        return np.ascontiguousarray(v.reshape(NH, HD).T)

    bq2 = two(inputs["bq"], perm)
    bk2 = two(inputs["bk"], perm)
    gq2 = two(inputs["gq"], perm)
    gk2 = two(inputs["gk"], perm)
    bv1 = np.asarray(inputs["bv"], np.float32).reshape(1, DIM).astype(_BF16)
    bo1 = np.asarray(inputs["bo"], np.float32).reshape(1, DIM).astype(_BF16)

    pswT = np.zeros((HD, HD), _BF16)
    for r in range(HD):
        pswT[(r + HALF) % HD, r] = 1.0   # lhsT of the half-swap permutation

    theta = _build_theta(freqs_angle, cs)              # [S, 64]
    cosT = np.cos(theta).T                             # [64, S]
    sinT = np.sin(theta).T
    cc2_full = np.concatenate([cosT, cosT], axis=0)    # [128, S]
    ss2_full = np.concatenate([-sinT, sinT], axis=0)

    pk = prior_k[p0:].reshape(n_prior, DIM)[:, perm]
    priorKT = np.zeros((DIM, np_pad), np.float32)
    priorKT[:, :n_prior] = pk.T
    priorKT = np.ascontiguousarray(priorKT.reshape(NH, HD, np_pad)).astype(_BF16)
    # priorVp: [NH, 128(p), NPT(lt), 130] partition-major dense grid with
    # the softmax-denominator ones column (masked on pad rows) at col 128
    pv = np.zeros((np_pad, NH, 130), np.float32)
    pv[:n_prior, :, :HD] = prior_v[p0:]
    pv[:n_prior, :, 128] = 1.0
    priorVp = np.ascontiguousarray(
        pv.reshape(NPT, 128, NH, 130).transpose(2, 1, 0, 3)).astype(_BF16)

    xT = np.ascontiguousarray(x[0].T).astype(_BF16)    # [DIM, S]

    NCT = -(-S_TOTAL // 128)
    PVALID = n_prior - 128 * (NPT - 1)
    CVALID = S_TOTAL - 128 * (NCT - 1)
    dmask = np.zeros((128, 2), _BF16)
    dmask[:PVALID, 0] = 1.0
    dmask[:CVALID, 1] = 1.0

    shared = dict(wq_p=wq_prep, wk_p=wk_prep, wv_p=wv_prep, wo_p=wo_prep,
                  bq2=bq2, bk2=bk2, gq2=gq2, gk2=gk2, bv1=bv1, bo1=bo1,
                  pswT=pswT, priorKT=priorKT, priorVp=priorVp, dmask=dmask)
    in_maps = []
    for c in range(NC):
        m = dict(shared)
        xc = xT[:, c * SC:(c + 1) * SC]                # [DIM, SC]
        m["xs_p"] = np.ascontiguousarray(
            xc.reshape(NK, 128, SC).transpose(1, 0, 2))
        m["cc2"] = np.ascontiguousarray(cc2_full[:, c * SC:(c + 1) * SC])
        m["ss2"] = np.ascontiguousarray(ss2_full[:, c * SC:(c + 1) * SC])
        in_maps.append(m)
    return in_maps, (n_prior,)


def kernel(**inputs) -> np.ndarray:
    import os
    from concourse.bass_utils import run_bass_kernel_spmd

    in_maps, key = _prep(inputs)
    if key not in _cache:
        _cache[key] = _build_program(*key)
    nc = _cache[key]

    trace = bool(int(os.environ.get("KERNEL_TRACE", "0")))
    try:
        res = run_bass_kernel_spmd(
            nc, in_maps, core_ids=list(range(NC)), trace=trace,
            trace_cores=list(range(NC)) if trace else None)
    except ModuleNotFoundError:
        res = run_bass_kernel_spmd(nc, in_maps, core_ids=list(range(NC)),
                                   trace=False)
    kernel.last_results = res
    outp = np.concatenate([res.results[c]["out"] for c in range(NC)], axis=0)
    return outp[None].astype(np.float32)


# revision 55
# speedup vs baseline: 1.3192x; 1.3192x over previous
"""Trainium2 Bass kernel for nn_CausalSelfAttention_5411658793445 (v2).

Token-sharded SPMD over 8 cores. Per core: 330 query tokens; K/V for the
current block exchanged via AllGather; every core attends over the full
kept window (2640 prior + 2640 current KV positions, dense l-tiling).

v2 structural changes vs v1:
  - exp batched over 3 l-tiles per ACTIVATE (amortizes ~300ns ACT overhead)
  - dense pass-2 l-grid (21 tiles, not 24): K gathered unpadded, V shards
    compacted into a partition-major dense DRAM grid after the AllGather
  - denominator via separate 1-column ones matmul with masked last-tile
    ones vector (no padding corrections anywhere)
  - V projection interleaved into pass-1 emission as PE filler so the
    tensor engine stays busy while ACT paces the exp stream
  - f32r (full-rate) matmuls for sum-of-squares and broadcast helpers
  - host-side weight/prior re-layout for contiguous, low-descriptor DMAs
  - output projection streamed directly at the tail, psum -> sbuf on ACT
"""

import math
from contextlib import ExitStack

import numpy as np
import ml_dtypes

NC = 8
DIM, NH, HD = 1536, 12, 128
HALF = 64
H, W = 22, 40
FRAME = H * W            # 880
S_TOTAL = 2640
SC = S_TOTAL // NC       # 330
ST = 110                 # token sub-chunk for [t,*] psum tiles (330 = 3*110)
NK = DIM // 128          # 12 contraction chunks
EPS = 1e-6
CT = 22
CH = 21
CW = 21
GRP = 3                  # l-tiles per exp group
KVW = 384                # per-core KV projection window (128-aligned)
NKC = -(-S_TOTAL // KVW)  # cores with real KV work (7)

_BF16 = ml_dtypes.bfloat16
_cache: dict = {}


def _build_theta(freqs_angle, cs):
    start_frame = cs // FRAME
    nf = S_TOTAL // FRAME
    t = freqs_angle[start_frame:start_frame + nf, :CT]
    h = freqs_angle[:H, CT:CT + CH]
    w = freqs_angle[:W, CT + CH:CT + CH + CW]
    tf = np.broadcast_to(t[:, None, None, :], (nf, H, W, CT))
    hf = np.broadcast_to(h[None, :, None, :], (nf, H, W, CH))
    wf = np.broadcast_to(w[None, None, :, :], (nf, H, W, CW))
    return np.concatenate([tf, hf, wf], axis=-1).reshape(nf * H * W, HALF)


def _v_pieces(shard):
    """Split the flat gathered-V token space into partition-aligned pieces.

    Returns [(c, tok0, n, p0, lt)]: shard c tokens [tok0, tok0+n) land at
    partitions [p0, p0+n) of l-tile lt of the dense [128, NCT, HD] grid.
    """
    pieces = []
    for c in range(NC):
        flat = c * shard
        left = shard
        tok = 0
        while left:
            p0 = flat % 128
            lt = flat // 128
            n = min(128 - p0, left)
            pieces.append((c, tok, n, p0, lt))
            flat += n
            tok += n
            left -= n
    return pieces


def _build_program(n_prior):
    import concourse.bass as bass  # noqa: F401
    import concourse.tile as tile
    from concourse import bacc, mybir
    from concourse.masks import make_identity

    f32 = mybir.dt.float32
    f32r = mybir.dt.float32r
    bf16 = mybir.dt.bfloat16
    Act = mybir.ActivationFunctionType
    Alu = mybir.AluOpType

    NPT = -(-n_prior // 128)         # prior l-tiles (21)
    NCT = -(-S_TOTAL // 128)         # current dense l-tiles (21)
    np_pad = NPT * 128
    PVALID = n_prior - 128 * (NPT - 1)   # valid rows in last prior tile (80)
    CVALID = S_TOTAL - 128 * (NCT - 1)   # valid rows in last current tile (80)
    PG = -(-NPT // GRP)              # prior exp groups per head (7)
    CG = -(-NCT // GRP)              # current exp groups per head (7)
    sm_scale = 1.0 / math.sqrt(HD)

    nc = bacc.Bacc("TRN2", target_bir_lowering=False, debug=False,
                   num_devices=NC)

    xs_p = nc.dram_tensor("xs_p", [128, NK, SC], bf16, kind="ExternalInput").ap()
    cc2 = nc.dram_tensor("cc2", [128, SC], f32, kind="ExternalInput").ap()
    ss2 = nc.dram_tensor("ss2", [128, SC], f32, kind="ExternalInput").ap()
    xkv_p = nc.dram_tensor("xkv_p", [128, NK, KVW], bf16,
                           kind="ExternalInput").ap()
    cckv = nc.dram_tensor("cckv", [128, KVW], f32, kind="ExternalInput").ap()
    sskv = nc.dram_tensor("sskv", [128, KVW], f32, kind="ExternalInput").ap()
    wq_p = nc.dram_tensor("wq_p", [NH, 128, NK, 128], bf16, kind="ExternalInput").ap()
    wk_p = nc.dram_tensor("wk_p", [NH, 128, NK, 128], bf16, kind="ExternalInput").ap()
    wv_p = nc.dram_tensor("wv_p", [128, NK, DIM], bf16, kind="ExternalInput").ap()
    wo_p = nc.dram_tensor("wo_p", [NH, 128, 3, 512], bf16, kind="ExternalInput").ap()
    bq2 = nc.dram_tensor("bq2", [HD, NH], f32, kind="ExternalInput").ap()
    bk2 = nc.dram_tensor("bk2", [HD, NH], f32, kind="ExternalInput").ap()
    gq2 = nc.dram_tensor("gq2", [HD, NH], f32, kind="ExternalInput").ap()
    gk2 = nc.dram_tensor("gk2", [HD, NH], f32, kind="ExternalInput").ap()
    bv1 = nc.dram_tensor("bv1", [1, DIM], bf16, kind="ExternalInput").ap()
    bo1 = nc.dram_tensor("bo1", [1, DIM], bf16, kind="ExternalInput").ap()
    pswT = nc.dram_tensor("pswT", [HD, HD], bf16, kind="ExternalInput").ap()
    priorKT = nc.dram_tensor("priorKT", [NH, HD, np_pad], bf16,
                             kind="ExternalInput").ap()
    priorVp = nc.dram_tensor("priorVp", [NH, 128, NPT, 130], bf16,
                             kind="ExternalInput").ap()
    dmask = nc.dram_tensor("dmask", [128, 2], bf16, kind="ExternalInput").ap()
    out = nc.dram_tensor("out", [SC, DIM], f32, kind="ExternalOutput").ap()
    import os
    DBG = bool(int(os.environ.get("KERNEL_DBG", "0")))
    if DBG:
        dbg_rrb = nc.dram_tensor("dbg_rrb", [128, SC], f32,
                                 kind="ExternalOutput").ap()
        dbg_esc = nc.dram_tensor("dbg_esc", [128, GRP * SC], f32,
                                 kind="ExternalOutput").ap()
        dbg_p1 = nc.dram_tensor("dbg_p1", [128, NH * 3 * 132], f32,
                                kind="ExternalOutput").ap()

    def r32(ap):
        return ap.bitcast(f32r)

    with tile.TileContext(nc, trace_sim=False) as tc, ExitStack() as ctx:
        consts = ctx.enter_context(tc.tile_pool(name="consts", bufs=1))
        wstr = ctx.enter_context(tc.tile_pool(name="wstr", bufs=2))
        xpool = ctx.enter_context(tc.tile_pool(name="xpool", bufs=1))
        acts = ctx.enter_context(tc.tile_pool(name="acts", bufs=1))
        sqp = ctx.enter_context(tc.tile_pool(name="sqp", bufs=2))
        escp = ctx.enter_context(tc.tile_pool(name="escp", bufs=2))
        kvs = ctx.enter_context(tc.tile_pool(name="kvs", bufs=2))
        g2p = ctx.enter_context(tc.tile_pool(name="g2p", bufs=2))
        smal = ctx.enter_context(tc.tile_pool(name="smal", bufs=2))
        dram = ctx.enter_context(tc.tile_pool(name="dram", bufs=1, space="DRAM"))
        # PSUM: big = 2 slots x 3 banks (score groups / proj accumulators /
        # tail out-proj chains); pone = pos accumulator (1 bank);
        # vone = V-filler chain / pss / transposes (1 bank, 2 slots... )
        big = ctx.enter_context(tc.tile_pool(name="big", bufs=2, space="PSUM"))
        pone = ctx.enter_context(tc.tile_pool(name="pone", bufs=1, space="PSUM"))
        vone = ctx.enter_context(tc.tile_pool(name="vone", bufs=1, space="PSUM"))

        # ---------------- constants ----------------
        _constv_cache = {}

        def constv(val):
            if val not in _constv_cache:
                t = consts.tile([128, 1], f32, name=f"cv_{len(_constv_cache)}")
                nc.vector.memset(t, val)
                _constv_cache[val] = t
            return _constv_cache[val]

        ident = consts.tile([128, 128], bf16)
        make_identity(nc, ident)
        ones_col_b = consts.tile([128, 1], bf16)
        nc.vector.memset(ones_col_b, 1.0)
        ones_row_b = consts.tile([1, 128], bf16)
        nc.vector.memset(ones_row_b, 1.0)
        ones_row_f = consts.tile([1, 128], f32)
        nc.vector.memset(ones_row_f, 1.0)
        dden_full = consts.tile([128, 1], bf16)
        nc.vector.memset(dden_full, 1.0)
        dmask_sb = consts.tile([128, 2], bf16)
        nc.sync.dma_start(dmask_sb, dmask)
        dden_p = dmask_sb[:, 0:1]
        dden_c = dmask_sb[:, 1:2]
        psw_sb = consts.tile([HD, HD], bf16)
        nc.sync.dma_start(psw_sb, pswT)
        cc_sb = consts.tile([128, SC], f32)
        ss_sb = consts.tile([128, SC], f32)
        nc.sync.dma_start(cc_sb, cc2)
        nc.sync.dma_start(ss_sb, ss2)
        bq_sb = consts.tile([HD, NH], f32)
        bk_sb = consts.tile([HD, NH], f32)
        gq_sb = consts.tile([HD, NH], f32)
        gk_sb = consts.tile([HD, NH], f32)
        nc.sync.dma_start(bq_sb, bq2)
        nc.sync.dma_start(bk_sb, bk2)
        nc.sync.dma_start(gq_sb, gq2)
        nc.sync.dma_start(gk_sb, gk2)
        bqg = consts.tile([HD, NH], f32)
        bkg = consts.tile([HD, NH], f32)
        nc.vector.tensor_mul(bqg, bq_sb, gq_sb)
        nc.vector.tensor_mul(bkg, bk_sb, gk_sb)
        bv_sb = consts.tile([1, DIM], bf16)
        bo_sb = consts.tile([1, DIM], bf16)
        nc.sync.dma_start(bv_sb, bv1)
        nc.sync.dma_start(bo_sb, bo1)
        zpad = consts.tile([64, DIM], bf16)
        nc.vector.memset(zpad, 0.0)

        # ---------------- x ----------------
        xs = xpool.tile([128, NK, SC], bf16)
        nc.sync.dma_start(xs, xs_p)
        xkv = xpool.tile([128, NK, KVW], bf16, tag="xkv")
        nc.sync.dma_start(xkv, xkv_p)
        cckv_sb = consts.tile([128, KVW], f32)
        sskv_sb = consts.tile([128, KVW], f32)
        nc.sync.dma_start(cckv_sb, cckv)
        nc.sync.dma_start(sskv_sb, sskv)

        # ---------------- internal DRAM ----------------
        k_cc_in = dram.tile([NH, HD, KVW], bf16)
        v_cc_in = dram.tile([KVW, DIM], bf16)
        kg = dram.tile([NC, NH, HD, KVW], bf16, addr_space="Shared")
        vg = dram.tile([NC, KVW, DIM], bf16, addr_space="Shared")
        rgroups = [list(range(NC))]

        # prefetch prior KV for head 0 (kvs pool) before any compute;
        # per-head prior loads issue from the Pool queue so AG-blocked DMAs
        # on SP never starve them.
        def load_prior(h):
            pkh = kvs.tile([128, np_pad], bf16, tag="kload", name=f"pk_{h}")
            nc.gpsimd.dma_start(pkh, priorKT[h])
            pvh = kvs.tile([128, NPT, 130], bf16, tag="vload", name=f"pv_{h}")
            nc.gpsimd.dma_start(pvh, priorVp[h])
            return pkh, pvh

        prior_bufs = {0: load_prior(0)}

        # ------------- q/k projection (dense) -------------
        def qk_projection(w_dram, b_sb, g_sb, bg_sb, name, x_t, width,
                          cc_t, ss_t):
            raw = acts.tile([128, NH, KVW], bf16, tag="raw",
                            name=f"raw_{name}")
            pss = vone.tile([128, 512], f32, tag="vch", name=f"pss_{name}")
            for m in range(NH):
                wm = wstr.tile([128, NK, 128], bf16, tag="wm",
                               name=f"wm_{name}_{m}")
                nc.sync.dma_start(wm, w_dram[m])
                ps = big.tile([128, GRP, 512], f32, tag="spp",
                              name=f"pj_{name}_{m}")
                for kk in range(NK):
                    nc.tensor.matmul(
                        ps[:, 0, :width], wm[:, kk, :], x_t[:, kk, :],
                        start=(kk == 0), stop=(kk == NK - 1))
                nc.scalar.activation(raw[:, m, :width], ps[:, 0, :width],
                                     Act.Identity,
                                     bias=bg_sb[:, m:m + 1],
                                     scale=g_sb[:, m:m + 1])
                sq = sqp.tile([128, KVW], bf16, tag="sq")
                nc.scalar.activation(sq[:, :width], ps[:, 0, :width],
                                     Act.Square, bias=b_sb[:, m:m + 1])
                nc.tensor.matmul(pss[0:1, :width], ones_col_b,
                                 sq[:, :width],
                                 start=(m == 0), stop=(m == NH - 1))
            r1 = smal.tile([1, KVW], f32, tag="r1")
            nc.scalar.activation(r1[:, :width], pss[0:1, :width], Act.Sqrt,
                                 scale=1.0 / DIM, bias=constv(EPS)[0:1])
            rr = smal.tile([1, KVW], f32, tag="rr")
            nc.vector.reciprocal(rr[:, :width], r1[:, :width])
            rrb = pone.tile([128, 3, 160], f32, tag="pos", name=f"rrb_{name}")
            rrf = rrb.rearrange("p a b -> p (a b)")
            nc.tensor.matmul(rrf[:, :width], ones_row_f, rr[:, :width],
                             start=True, stop=True)
            ccr = sqp.tile([128, KVW], f32, tag="ccr", name=f"ccr_{name}")
            ssr = sqp.tile([128, KVW], f32, tag="ssr", name=f"ssr_{name}")
            nc.vector.tensor_mul(ccr[:, :width], cc_t, rrf[:, :width])
            nc.vector.tensor_mul(ssr[:, :width], ss_t, rrf[:, :width])
            if DBG and name == "q":
                nc.sync.dma_start(dbg_rrb, rrf[:, :SC])
            return raw, ccr, ssr

        def rope_chunk(raw, ccr, ssr, m, dst_ap, name, width):
            # dst = raw*ccr + swap_halves(raw)*ssr (swap via PE matmul)
            pw = big.tile([128, GRP, 512], f32, tag="spp", name=f"sw_{name}_{m}")
            nc.tensor.matmul(pw[:, 0, :width], psw_sb, raw[:, m, :width],
                             start=True, stop=True)
            m1 = sqp.tile([128, KVW], f32, tag="m1")
            nc.vector.tensor_mul(m1[:, :width], raw[:, m, :width],
                                 ccr[:, :width])
            m2 = sqp.tile([128, KVW], f32, tag="m2")
            nc.vector.tensor_mul(m2[:, :width], pw[:, 0, :width],
                                 ssr[:, :width])
            nc.vector.tensor_add(dst_ap, m1[:, :width], m2[:, :width])

        # ---------- K projection + AllGather ----------
        raw_k, ccr_k, ssr_k = qk_projection(wk_p, bk_sb, gk_sb, bkg, "k",
                                            xkv, KVW, cckv_sb, sskv_sb)
        kn = acts.tile([128, NH, KVW], bf16, tag="kn")
        for m in range(NH):
            rope_chunk(raw_k, ccr_k, ssr_k, m, kn[:, m, :], "k", KVW)
        for m in range(NH):
            nc.gpsimd.dma_start(k_cc_in[m], kn[:, m, :])
        nc.gpsimd.collective_compute(
            "AllGather", Alu.bypass, replica_groups=rgroups,
            ins=[k_cc_in.opt()], outs=[kg.opt()])

        # ---------- V projection (dense, right after K so AG-V starts
        # early) -- direct [t, d] production in three 128-row token chunks
        # of the aligned KV window, then contiguous shard DMAs + AllGather.
        wvs_all = xpool.tile([128, NK, DIM], bf16, tag="wvs")
        nc.sync.dma_start(wvs_all, wv_p)
        vt = acts.tile([128, 3, DIM], bf16, tag="vt")
        for oc in range(3):
            for tc in range(3):
                pv = vone.tile([128, 512], f32, tag="vch",
                               name=f"pv_{oc}_{tc}")
                for kk in range(NK):
                    nc.tensor.matmul(
                        pv[:, :], xkv[:, kk, tc * 128:(tc + 1) * 128],
                        wvs_all[:, kk, oc * 512:(oc + 1) * 512],
                        start=(kk == 0), stop=False)
                nc.tensor.matmul(pv[:, :], ones_row_b,
                                 bv_sb[:, oc * 512:(oc + 1) * 512],
                                 start=False, stop=True)
                nc.vector.tensor_copy(
                    vt[:, tc, oc * 512:(oc + 1) * 512], pv[:, :])
        for tc in range(3):
            nc.gpsimd.dma_start(v_cc_in[tc * 128:(tc + 1) * 128, :],
                                vt[:, tc, :])
        nc.gpsimd.collective_compute(
            "AllGather", Alu.bypass, replica_groups=rgroups,
            ins=[v_cc_in.opt()], outs=[vg.opt()])

        # ---------- Q projection ----------
        raw_q, ccr_q, ssr_q = qk_projection(wq_p, bq_sb, gq_sb, bqg, "q",
                                            xs, SC, cc_sb, ss_sb)
        qn = acts.tile([128, NH, SC], bf16, tag="qn")
        for m in range(NH):
            rope_chunk(raw_q, ccr_q, ssr_q, m, qn[:, m, :], "q", SC)
        v_units = []

        # wo fully resident (reuses the wv slot; loads during pass-1)
        wo_all = xpool.tile([128, NH, 3, 512], bf16, tag="wvs", name="wo_all")
        nc.sync.dma_start(wo_all, wo_p.rearrange("h ki oc c -> ki h oc c"))

        # ---------- attention ----------
        part1 = acts.tile([128, NH, 3, 132], bf16, tag="part1")
        oT = acts.tile([128, NH, SC], bf16, tag="oT")

        def attn_all(specs, fillers, phase):
            """Run one softmax pass over all heads, software-pipelined.

            Scores+exp of pipeline step i+1 are emitted before the AV
            matmuls of step i, across head boundaries, so the in-order PE
            queue always has exp-independent work ahead of each
            exp-dependent AV block. `fillers` are exp-free PE work units
            popped one per step to absorb the ACT-vs-PE rate gap.
            """
            flat = []
            for sp in specs:
                for g in range(sp["n_groups"]):
                    flat.append((sp, g))
            esc_q = []

            def emit_scores(i):
                sp, g = flat[i]
                g0 = g * GRP
                gn = min(GRP, sp["n_tiles"] - g0)
                spp = big.tile([128, GRP, 512], f32, tag="spp",
                               name=f"sc_{phase}_{sp['h']}_{g}")
                for j in range(gn):
                    nc.tensor.matmul(spp[:, j, :SC], sp["k_tile"](g0 + j),
                                     qn[:, sp["h"], :], start=True, stop=True)
                esc = escp.tile([128, GRP, SC], bf16, tag="esc")
                nc.scalar.activation(esc[:, :gn, :], spp[:, :gn, :SC],
                                     Act.Exp, scale=float(sm_scale))
                if DBG and phase == "p" and i == 0:
                    dbge = acts.tile([128, GRP * SC], f32, tag="dbge")
                    nc.vector.tensor_copy(
                        dbge.rearrange("p (a b) -> p a b", a=GRP),
                        esc)
                    nc.sync.dma_start(dbg_esc, dbge)
                esc_q.append((sp, g, esc))

            def emit_av():
                sp, g, esc = esc_q.pop(0)
                if g == 0:
                    # prefetch the next head's KV now: the previous head's
                    # reads are all emitted, so the pool slot is reclaimable
                    if sp.get("pre") is not None:
                        sp["pre"]()
                    sp["pos"] = pone.tile([128, 3, 160], f32, tag="pos",
                                          name=f"pos_{phase}_{sp['h']}")
                pos = sp["pos"]
                g0 = g * GRP
                gn = min(GRP, sp["n_tiles"] - g0)
                for j in range(gn):
                    lt = g0 + j
                    last = lt == sp["n_tiles"] - 1
                    vt_ap = sp["v_tile"](lt)   # [128, 129]: V | ones-col
                    for si in range(3):
                        # a psum-bank `start` zeroes the whole bank, so only
                        # the very first matmul of the bank may set it
                        nc.tensor.matmul(
                            pos[:ST, si, 0:129],
                            esc[:, j, si * ST:(si + 1) * ST], vt_ap,
                            start=(lt == 0 and si == 0), stop=last,
                            skip_group_check=True)
                if g == sp["n_groups"] - 1 and sp.get("post") is not None:
                    sp["post"](sp)

            emit_scores(0)
            for i in range(1, len(flat) + 1):
                if fillers:
                    fillers.pop(0)()
                if i < len(flat):
                    emit_scores(i)
                emit_av()

        # ---- pass 1: prior KV (AllGathers in flight) ----
        def p1_pre(h):
            def f():
                if h + 1 < NH:
                    prior_bufs[h + 1] = load_prior(h + 1)
            return f

        def p1_post(sp):
            pos = sp["pos"]
            h = sp["h"]
            for si in range(3):
                nc.vector.tensor_copy(part1[:ST, h, si, 0:129],
                                      pos[:ST, si, 0:129])

        p1_specs = []
        for h in range(NH):
            p1_specs.append(dict(
                h=h, n_tiles=NPT, n_groups=PG,
                k_tile=lambda lt, h=h: prior_bufs[h][0][:, lt * 128:
                                                        (lt + 1) * 128],
                v_tile=lambda lt, h=h: prior_bufs[h][1][:, lt, 0:129],
                pre=p1_pre(h), post=p1_post))
        attn_all(p1_specs, v_units, "p")
        if DBG:
            nc.sync.dma_start(
                dbg_p1, part1.rearrange("p a b c -> p (a b c)"))
        # drain any remaining V units (ensures AG-V is triggered)
        while v_units:
            v_units.pop(0)()

        # ---- pass 2: gathered current KV (dense, 128-aligned shards) ----
        def load_g2(h):
            kgh = g2p.tile([128, NKC * KVW], bf16, tag="kload2",
                           name=f"kg_{h}")
            nc.sync.dma_start(
                kgh.rearrange("p (c t) -> p c t", c=NKC),
                kg[0:NKC, h].rearrange("c p t -> p c t"))
            if NKC * KVW > S_TOTAL:
                nc.vector.memset(kgh[:, S_TOTAL:], 0.0)
            vgh = g2p.tile([128, NCT, 130], bf16, tag="vload2",
                           name=f"vg_{h}")
            nc.sync.dma_start(
                vgh[:, :, 0:HD].rearrange("p (c j) d -> p c j d", c=NKC),
                vg[0:NKC, :, h * HD:(h + 1) * HD].rearrange(
                    "c (j p) d -> p c j d", p=128))
            nc.vector.memset(vgh[:, :, 128:129], 1.0)
            # zero the pad rows (tokens >= S_TOTAL) of the last tile,
            # including the ones column -- DMA writes may start at any
            # partition, unlike engine ops
            if CVALID < 128:
                nc.sync.dma_start(vgh[CVALID:128, NCT - 1, :],
                                  zpad[0:128 - CVALID, 0:130])
            return kgh, vgh

        g2_bufs = {0: load_g2(0)}

        def p2_pre(h):
            def f():
                if h + 1 < NH:
                    g2_bufs[h + 1] = load_g2(h + 1)
            return f

        def p2_post(sp):
            # combine partials + divide (all reads of pos first, since the
            # transpose psum reuses the pos slot), then transpose to [d, t]
            pos = sp["pos"]
            h = sp["h"]
            odivs = []
            for si in range(3):
                nsb = smal.tile([128, 132], f32, tag="nsb")
                nc.vector.tensor_add(nsb[:ST, 0:129], pos[:ST, si, 0:129],
                                     part1[:ST, h, si, 0:129])
                rcp = smal.tile([128, 1], f32, tag="rcp")
                nc.vector.reciprocal(rcp[:ST, :], nsb[:ST, 128:129])
                odiv = smal.tile([128, 128], bf16, tag="odiv",
                                 name=f"odiv_{h}_{si}", bufs=4)
                nc.scalar.activation(odiv[:ST, :], nsb[:ST, 0:128],
                                     Act.Copy, scale=rcp[:ST, 0:1])
                odivs.append(odiv)
            for si in range(3):
                ptr = pone.tile([128, 3, 160], bf16, tag="pos",
                                name=f"ptr_{h}_{si}")
                ptrf = ptr.rearrange("p a b -> p (a b)")
                nc.tensor.transpose(ptrf[:, :ST], odivs[si][:ST, :],
                                    ident[:ST, :ST])
                nc.vector.tensor_copy(oT[:, h, si * ST:(si + 1) * ST],
                                      ptrf[:, :ST])

        p2_specs = []
        for h in range(NH):
            p2_specs.append(dict(
                h=h, n_tiles=NCT, n_groups=CG,
                k_tile=lambda lt, h=h: g2_bufs[h][0][:, lt * 128:
                                                     (lt + 1) * 128],
                v_tile=lambda lt, h=h: g2_bufs[h][1][:, lt, 0:129],
                pre=p2_pre(h), post=p2_post))
        attn_all(p2_specs, [], "c")

        # ---------- output projection (tail, weights resident) ----------
        for oc in range(3):
            po = big.tile([128, GRP, 512], f32, tag="spp", name=f"po_{oc}")
            for h in range(NH):
                for tci in range(3):
                    nc.tensor.matmul(
                        po[:ST, tci, :], oT[:, h, tci * ST:(tci + 1) * ST],
                        wo_all[:, h, oc, :], start=(h == 0), stop=False)
            for tci in range(3):
                nc.tensor.matmul(
                    po[:ST, tci, :], ones_row_b[:, :ST],
                    bo_sb[:, oc * 512:(oc + 1) * 512],
                    start=False, stop=True)
            for tci in range(3):
                ob = sqp.tile([128, 512], f32, tag="ob",
                              name=f"ob_{oc}_{tci}")
                nc.scalar.activation(ob[:ST, :], po[:ST, tci, :], Act.Copy)
                nc.sync.dma_start(
                    out[tci * ST:(tci + 1) * ST, oc * 512:(oc + 1) * 512],
                    ob[:ST, :])

    nc.compile()
    return nc


def _prep(inputs):
    x = np.asarray(inputs["x"], np.float32)
    freqs_angle = np.asarray(inputs["freqs_angle"], np.float32)
    prior_k = np.asarray(inputs["prior_k"], np.float32)
    prior_v = np.asarray(inputs["prior_v"], np.float32)
    cs = int(np.asarray(inputs["current_start"]))

    block = 3 * FRAME
    block_end = (cs // block + 1) * block
    keep_from = max(0, block_end - 6 * FRAME)
    keep_size = min(cs + S_TOTAL - keep_from, prior_k.shape[0] + S_TOTAL)
    n_prior = keep_size - S_TOTAL
    p0 = prior_k.shape[0] - n_prior
    NPT = -(-n_prior // 128)
    np_pad = NPT * 128

    perm = np.concatenate(
        [h * HD + np.concatenate([np.arange(0, HD, 2), np.arange(1, HD, 2)])
         for h in range(NH)])

    Wq = np.asarray(inputs["Wq"], np.float32)[perm]
    Wk = np.asarray(inputs["Wk"], np.float32)[perm]
    Wv = np.asarray(inputs["Wv"], np.float32)
    Wo = np.asarray(inputs["Wo"], np.float32)

    # wq_p / wk_p: [NH(m), 128(ki), NK(ko), 128(col)]; W row=outdim col=indim
    # device matmul: stationary wm[:, kk, :] = W^T chunk [ki, m-cols]
    def qk_prep(Wm):
        WT = Wm.T.reshape(NK, 128, NH, 128)           # [ko, ki, m, col]
        return np.ascontiguousarray(
            WT.transpose(2, 1, 0, 3)).astype(_BF16)    # [m, ki, ko, col]

    wq_prep = qk_prep(Wq)
    wk_prep = qk_prep(Wk)
    # wv_p: [128(ki), NK(ko), 512]; chunk for oc: [:, kk, oc*512...] is
    # WvT[kk*128:(kk+1)*128 rows?? -> WvT reshaped
    WvT = Wv.T.reshape(NK, 128, DIM)                   # [ko, ki, outcol]
    wv_prep = np.ascontiguousarray(
        WvT.transpose(1, 0, 2)).astype(_BF16)          # [ki, ko, 1536]
    WoT = Wo.T.reshape(NK, 128, 3, 512)                # [h, ki, oc, col]
    wo_prep = np.ascontiguousarray(
        WoT.transpose(0, 1, 2, 3)).astype(_BF16)       # [h(=ko), ki, oc, col]

    def two(vec, p=None):
        v = np.asarray(vec, np.float32)
        if p is not None:
            v = v[p]
        return np.ascontiguousarray(v.reshape(NH, HD).T)

    bq2 = two(inputs["bq"], perm)
    bk2 = two(inputs["bk"], perm)
    gq2 = two(inputs["gq"], perm)
    gk2 = two(inputs["gk"], perm)
    bv1 = np.asarray(inputs["bv"], np.float32).reshape(1, DIM).astype(_BF16)
    bo1 = np.asarray(inputs["bo"], np.float32).reshape(1, DIM).astype(_BF16)

    pswT = np.zeros((HD, HD), _BF16)
    for r in range(HD):
        pswT[(r + HALF) % HD, r] = 1.0   # lhsT of the half-swap permutation

    theta = _build_theta(freqs_angle, cs)              # [S, 64]
    cosT = np.cos(theta).T                             # [64, S]
    sinT = np.sin(theta).T
    cc2_full = np.concatenate([cosT, cosT], axis=0)    # [128, S]
    ss2_full = np.concatenate([-sinT, sinT], axis=0)

    pk = prior_k[p0:].reshape(n_prior, DIM)[:, perm]
    priorKT = np.zeros((DIM, np_pad), np.float32)
    priorKT[:, :n_prior] = pk.T
    priorKT = np.ascontiguousarray(priorKT.reshape(NH, HD, np_pad)).astype(_BF16)
    # priorVp: [NH, 128(p), NPT(lt), 130] partition-major dense grid with
    # the softmax-denominator ones column (masked on pad rows) at col 128
    pv = np.zeros((np_pad, NH, 130), np.float32)
    pv[:n_prior, :, :HD] = prior_v[p0:]
    pv[:n_prior, :, 128] = 1.0
    priorVp = np.ascontiguousarray(
        pv.reshape(NPT, 128, NH, 130).transpose(2, 1, 0, 3)).astype(_BF16)

    xT = np.ascontiguousarray(x[0].T).astype(_BF16)    # [DIM, S]

    NCT = -(-S_TOTAL // 128)
    PVALID = n_prior - 128 * (NPT - 1)
    CVALID = S_TOTAL - 128 * (NCT - 1)
    dmask = np.zeros((128, 2), _BF16)
    dmask[:PVALID, 0] = 1.0
    dmask[:CVALID, 1] = 1.0

    shared = dict(wq_p=wq_prep, wk_p=wk_prep, wv_p=wv_prep, wo_p=wo_prep,
                  bq2=bq2, bk2=bk2, gq2=gq2, gk2=gk2, bv1=bv1, bo1=bo1,
                  pswT=pswT, priorKT=priorKT, priorVp=priorVp, dmask=dmask)
    xT_pad = np.zeros((DIM, NC * KVW), _BF16)
    xT_pad[:, :S_TOTAL] = xT
    cc_pad = np.zeros((128, NC * KVW), np.float32)
    ss_pad = np.zeros((128, NC * KVW), np.float32)
    cc_pad[:, :S_TOTAL] = cc2_full
    ss_pad[:, :S_TOTAL] = ss2_full

    in_maps = []
    for c in range(NC):
        m = dict(shared)
        xc = xT[:, c * SC:(c + 1) * SC]                # [DIM, SC]
        m["xs_p"] = np.ascontiguousarray(
            xc.reshape(NK, 128, SC).transpose(1, 0, 2))
        m["cc2"] = np.ascontiguousarray(cc2_full[:, c * SC:(c + 1) * SC])
        m["ss2"] = np.ascontiguousarray(ss2_full[:, c * SC:(c + 1) * SC])
        xkc = xT_pad[:, c * KVW:(c + 1) * KVW]         # [DIM, KVW]
        m["xkv_p"] = np.ascontiguousarray(
            xkc.reshape(NK, 128, KVW).transpose(1, 0, 2))
        m["cckv"] = np.ascontiguousarray(cc_pad[:, c * KVW:(c + 1) * KVW])
        m["sskv"] = np.ascontiguousarray(ss_pad[:, c * KVW:(c + 1) * KVW])
        in_maps.append(m)
    return in_maps, (n_prior,)


def kernel(**inputs) -> np.ndarray:
    import os
    from concourse.bass_utils import run_bass_kernel_spmd

    in_maps, key = _prep(inputs)
    if key not in _cache:
        _cache[key] = _build_program(*key)
    nc = _cache[key]

    trace = bool(int(os.environ.get("KERNEL_TRACE", "0")))
    try:
        res = run_bass_kernel_spmd(
            nc, in_maps, core_ids=list(range(NC)), trace=trace,
            trace_cores=list(range(NC)) if trace else None)
    except ModuleNotFoundError:
        res = run_bass_kernel_spmd(nc, in_maps, core_ids=list(range(NC)),
                                   trace=False)
    kernel.last_results = res
    outp = np.concatenate([res.results[c]["out"] for c in range(NC)], axis=0)
    return outp[None].astype(np.float32)
